# revision 13
# baseline (speedup 1.0000x reference)
"""Trainium2 Bass kernel for nn_Attention_7679401525457.

score_i = relu(Linear(tanh(concat(h_i, z)))); alphas = softmax(scores);
attention = sum_i alphas_i * h_i.

Data-parallel over 8 NeuronCores: batch dim (32) sharded 4-per-core; the
tiny W/b replicated. Each core reads its encoder slice from HBM exactly
once in ~1 MiB chunks and computes behind the DMA stream.

Performance structure (v2, "onedma"):
- The 18 chunk DMAs own the SP HWDGE ring exclusively. The two HWDGE
  rings share one descriptor-generation RTL block, so every extra DMA
  mid-stream stalls the input stream; the kernel therefore emits exactly
  ONE output DMA ([4, 1056] on the ACT ring) instead of the previous 9
  (measured ~4.7us of stream stall from those).
- All four batches accumulate their weighted sums into one shared PSUM
  region: each matmul's stationary is a [128, 4] slice whose column b
  holds batch b's alphas and the rest zeros, so every mm writes all 4
  output rows (others +0) and a single start/stop pair brackets the
  whole accumulation. Softmax denominators ride the same trick via a
  ones-column stationary into a second PSUM region.
- Alphas stay unnormalized (relu keeps scores >= 0 so exp is bounded);
  per-batch denominators ship with the output rows in the single
  [4, 1024+32] output tensor and the host finishes the divide.
- Chunk pipeline keeps a one-chunk lookahead: ACT's stream is tanh(k),
  tanh(k+1), exp(k), ... so exp (which needs DVE's score reduction of
  chunk k) doesn't stall ACT. The last batch streams its final tile as
  two 512-column pieces to shorten the post-stream dependency tail.
- DVE is marginally co-critical with the stream (measured: 32 STT x
  1.33us + 16 relu x 0.31us ~= 47.4us busy vs 48us stream), so relu/exp
  run at batch granularity for all but the last batch (batchexp) to
  shave ~3.7us of DVE busy; the last batch stays per-chunk to keep the
  tail chain short.
"""

import numpy as np

import concourse.bass as bass
import concourse.bacc as bacc
import concourse.mybir as mybir
import concourse.tile as tile
from concourse.bass_utils import run_bass_kernel_spmd

B, S, D = 32, 1024, 1024
NCORES = 8
BPC = B // NCORES  # batches per core
NT = S // 128  # s-tiles per batch
F32 = mybir.dt.float32
F32R = mybir.dt.float32r
BF16 = mybir.dt.bfloat16
AF = mybir.ActivationFunctionType
ALU = mybir.AluOpType

TAILQ = 2       # splits of the very last tile
OW = D + 4 * NT  # output row: [row 1024 | denominator tiles 32]

_CACHE = {}


LASTW = None  # if set, batch 3 streams tile 7 cols [LASTW:1024] early and
              # the [0:LASTW] sliver as the very last DMA (short tail chain)


def _chunk_plan(b):
    # entries: (tile_start, n_tiles, dh); dh None = full tile rows,
    # int 0..TAILQ-1 = sequential 1024/TAILQ-column slice of a single tile,
    # tuple (slot, c0, c1) = explicit column range (reordered tail scheme)
    if b < BPC - 1:
        return [(0, 2, None), (2, 2, None), (4, 2, None), (6, 2, None)]
    if LASTW is not None:
        return [
            (0, 2, None), (7, 1, (0, LASTW, D)), (2, 2, None),
            (4, 2, None), (6, 1, None), (7, 1, (1, 0, LASTW)),
        ]
    return [(0, 2, None), (2, 2, None), (4, 2, None), (6, 1, None)] + [
        (7, 1, q) for q in range(TAILQ)
    ]


def _is_lastpiece(dh):
    if dh is None:
        return None
    return dh[0] == 1 if isinstance(dh, tuple) else dh == TAILQ - 1


def _chunks():
    return [(b, t0, nt, dh) for b in range(BPC) for (t0, nt, dh) in _chunk_plan(b)]


def _build(loop_R=None, hp_tail=False, relu2=False, tailq=None, looka=1,
           hp_epi=False, bodyx=1, stag=False, dualq=False, lastw=None,
           encb=4, ttb=3, junkb=2, batchexp=True, totfirst=True):
    global TAILQ, LASTW
    if tailq is not None:
        TAILQ = tailq
    LASTW = lastw
    # loop_R: if set, wrap the pipeline in a hardware loop repeated
    # loop_R times (timing harness only; output unchanged).
    nc = bacc.Bacc("TRN2", target_bir_lowering=False, debug=False)

    enc = nc.dram_tensor("enc", [BPC, S, D], F32, kind="ExternalInput")
    # zt[p, b*8+c] = z[b, p*8+c]   (z = decoder_hidden[-1] core slice)
    zt = nc.dram_tensor("zt", [128, BPC * 8], F32, kind="ExternalInput")
    w1rep = nc.dram_tensor("w1rep", [128, D], BF16, kind="ExternalInput")
    # w2t[p, c] = W2[p*8+c]
    w2t = nc.dram_tensor("w2t", [128, 8], F32, kind="ExternalInput")
    # bb128 = b[0]/128 replicated, so a ones-matmul partition-sum adds b[0]
    bb128 = nc.dram_tensor("bb128", [128, 1], F32, kind="ExternalInput")
    attx = nc.dram_tensor("attx", [BPC, OW], F32, kind="ExternalOutput")

    with tile.TileContext(nc) as tc:
        with (
            tc.tile_pool(name="const", bufs=1) as cpool,
            tc.tile_pool(name="encp", bufs=encb) as encp,
            tc.tile_pool(name="ttp", bufs=ttb) as ttp,
            tc.tile_pool(name="junkp", bufs=junkb) as junkp,
            tc.tile_pool(name="smallp", bufs=2) as smallp,
            tc.tile_pool(name="asmp", bufs=2) as asmp,
            tc.tile_pool(name="pscb", bufs=1, space="PSUM") as pscb,
            tc.tile_pool(name="psg", bufs=1, space="PSUM") as psg,
        ):
            # ---- constants (GPSIMD SWDGE ring; keeps HW rings clear) ----
            w1t = cpool.tile([128, D], BF16)
            nc.gpsimd.dma_start(w1t[:], w1rep.ap())
            ztt = cpool.tile([128, BPC * 8], F32)
            nc.gpsimd.dma_start(ztt[:], zt.ap())
            w2tt = cpool.tile([128, 8], F32)
            nc.gpsimd.dma_start(w2tt[:], w2t.ap())
            bbt = cpool.tile([128, 1], F32)
            nc.gpsimd.dma_start(bbt[:], bb128.ap())
            ones128 = cpool.tile([128, 1], F32)
            nc.vector.memset(ones128[:], 1.0)
            ones_sq = cpool.tile([128, 128], F32)
            nc.vector.memset(ones_sq[:], 1.0)

            # ---- prepass: cb[:, b] = tanh(z_b) @ W2 + b0, on all partitions
            tz = cpool.tile([128, BPC * 8], F32)
            nc.scalar.activation(tz[:], ztt[:], AF.Tanh)
            czp = cpool.tile([128, BPC], F32)
            zjunk = cpool.tile([128, 8], F32)
            for bi in range(BPC):
                nc.vector.scalar_tensor_tensor(
                    out=zjunk[:],
                    in0=tz[:, bi * 8 : (bi + 1) * 8],
                    scalar=1.0,
                    in1=w2tt[:],
                    op0=ALU.mult,
                    op1=ALU.mult,
                    accum_out=czp[:, bi : bi + 1],
                )
            czp2 = cpool.tile([128, BPC], F32)
            nc.vector.tensor_scalar_add(czp2[:], czp[:], bbt[:, 0:1])
            cb_ps = pscb.tile([128, BPC], F32)
            nc.tensor.matmul(cb_ps[:], ones_sq[:], czp2[:], start=True, stop=True)
            cb = cpool.tile([128, BPC], F32)
            nc.scalar.copy(cb[:], cb_ps[:])

            # ---- shared PSUM accumulators: row b = batch b ----
            ap0g = psg.tile([BPC, 512], F32, tag="ap0g", name="ap0g")
            ap1g = psg.tile([BPC, 512], F32, tag="ap1g", name="ap1g")
            totg = psg.tile([BPC, 4 * NT], F32, tag="totg", name="totg")
            # ones_pad[:, b*4:(b+1)*4] has ones in column b, zeros elsewhere
            ones_pad = cpool.tile([128, 4 * BPC], F32)
            nc.vector.memset(ones_pad[:], 0.0)
            for bi in range(BPC):
                nc.vector.memset(ones_pad[:, bi * 4 + bi : bi * 4 + bi + 1], 1.0)

            # persistent per-batch alpha tiles, zeroed once; each batch's
            # exp only writes its own column of each 4-wide group, so the
            # zeros in the other columns survive reuse across iterations.
            alp_tiles = []
            for bi in range(BPC):
                t_ = smallp.tile(
                    [128, 4 * NT], F32R, tag=f"alp{bi}", name=f"alp{bi}"
                )
                nc.vector.memset(t_[:].bitcast(F32), 0.0)
                alp_tiles.append(t_)

            state = {}
            CH = _chunks()
            NTOT = len(CH)

            def load_chunk(k):
                """DMA chunk k, tanh it, fused mult+reduce scores, relu."""
                b, t0, ntl, dh = CH[k]
                if t0 == 0:
                    st = {}
                    st["encT"] = encp.tile(
                        [128, NT * D], F32R, tag="enc", name="encT"
                    )
                    st["src"] = enc.ap()[b].rearrange(
                        "(t p) d -> p t d", p=128
                    ).bitcast(F32R)
                    st["sc"] = smallp.tile([128, NT], F32, tag="sc", name="sc")
                    st["sr"] = smallp.tile([128, NT], F32, tag="sr", name="sr")
                    st["alp"] = alp_tiles[b]
                    state[b] = st
                st = state[b]
                if dh is not None:
                    # split-tile chunk: DMA + tanh + partial dot for one
                    # column slice of tile t0
                    if isinstance(dh, tuple):
                        slot, c0, c1 = dh
                    else:
                        w = D // TAILQ
                        slot, c0, c1 = dh, dh * w, (dh + 1) * w
                    w = c1 - c0
                    nc.sync.dma_start(
                        st["encT"][:, t0 * D + c0 : t0 * D + c1],
                        st["src"][:, t0, c0:c1],
                    )
                    from contextlib import nullcontext
                    hpc = tc.high_priority() if hp_tail else nullcontext()
                    with hpc:
                        tt = ttp.tile([128, 2 * D], BF16, tag="tt", name="tt")
                        nc.scalar.activation(
                            tt[:, 0:w],
                            st["encT"][:, t0 * D + c0 : t0 * D + c1].bitcast(F32),
                            AF.Tanh,
                        )
                        if slot == 0:
                            st["schalf"] = smallp.tile(
                                [128, 2 if isinstance(dh, tuple) else TAILQ],
                                F32, tag="schalf", name="schalf",
                            )
                        junk = junkp.tile([128, D], F32, tag="junk", name="junk")
                        nc.vector.scalar_tensor_tensor(
                            out=junk[:, 0:w],
                            in0=tt[:, 0:w],
                            scalar=1.0,
                            in1=w1t[:, c0:c1],
                            op0=ALU.mult,
                            op1=ALU.mult,
                            accum_out=st["schalf"][:, slot : slot + 1],
                        )
                    if _is_lastpiece(dh):
                        with (tc.high_priority() if hp_tail else nullcontext()):
                            # combine partials, then relu(score + cb)
                            nc.vector.tensor_reduce(
                                out=st["sc"][:, t0 : t0 + 1],
                                in_=st["schalf"][:],
                                axis=mybir.AxisListType.X,
                                op=ALU.add,
                            )
                            nc.vector.tensor_scalar(
                                out=st["sr"][:, t0 : t0 + 1],
                                in0=st["sc"][:, t0 : t0 + 1],
                                scalar1=cb[:, b : b + 1],
                                scalar2=0.0,
                                op0=ALU.add,
                                op1=ALU.max,
                            )
                    return
                dma_eng = nc.gpsimd if (dualq and (k % 2)) else nc.sync
                dma_eng.dma_start(
                    st["encT"][:, t0 * D : (t0 + ntl) * D].rearrange(
                        "p (t d) -> p t d", t=ntl
                    ),
                    st["src"][:, t0 : t0 + ntl, :],
                )
                tt = ttp.tile([128, 2 * D], BF16, tag="tt", name="tt")
                nc.scalar.activation(
                    tt[:, 0 : ntl * D],
                    st["encT"][:, t0 * D : (t0 + ntl) * D].bitcast(F32),
                    AF.Tanh,
                )
                for kk in range(ntl):
                    t = t0 + kk
                    junk = junkp.tile([128, D], F32, tag="junk", name="junk")
                    # fused multiply+row-sum: out=(tt*1)*w1, accum=sum
                    # (tensor_tensor_reduce crashes the exec unit on this
                    # runtime; scalar_tensor_tensor accum works)
                    nc.vector.scalar_tensor_tensor(
                        out=junk[:],
                        in0=tt[:, kk * D : (kk + 1) * D],
                        scalar=1.0,
                        in1=w1t[:],
                        op0=ALU.mult,
                        op1=ALU.mult,
                        accum_out=st["sc"][:, t : t + 1],
                    )
                if batchexp and b < BPC - 1:
                    return  # relu/exp deferred to the batch boundary
                if relu2 and b < BPC - 1:
                    if t0 % 4 != 2:
                        return
                    cols = slice(t0 - 2, t0 + 2)
                else:
                    cols = slice(t0, t0 + ntl)
                nc.vector.tensor_scalar(
                    out=st["sr"][:, cols],
                    in0=st["sc"][:, cols],
                    scalar1=cb[:, b : b + 1],
                    scalar2=0.0,
                    op0=ALU.add,
                    op1=ALU.max,
                )

            def exp_mm_chunk(k):
                """exp chunk k's scores into the batch's column of alp,
                accumulate its weighted sum into the shared PSUM rows."""
                b, t0, ntl, dh = CH[k]
                if dh is not None and not _is_lastpiece(dh):
                    return  # partial split-tile: nothing to exp yet
                st = state[b]
                if batchexp and b < BPC - 1:
                    if t0 + ntl != NT:
                        return
                    nc.vector.tensor_scalar(
                        out=st["sr"][:], in0=st["sc"][:],
                        scalar1=cb[:, b : b + 1], scalar2=0.0,
                        op0=ALU.add, op1=ALU.max,
                    )
                    t0, ntl = 0, NT
                if relu2 and b < BPC - 1:
                    if t0 % 4 != 2:
                        return
                    t0, ntl = t0 - 2, 4
                alp3 = st["alp"][:].rearrange("p (t c) -> p t c", c=4)
                hp = hp_tail and b == BPC - 1
                from contextlib import nullcontext
                with tc.high_priority() if hp else nullcontext():
                    nc.scalar.activation(
                        alp3[:, t0 : t0 + ntl, b],
                        st["sr"][:, t0 : t0 + ntl],
                        AF.Exp,
                    )
                first = b == 0 and t0 == 0
                lastb = b == BPC - 1
                if totfirst and lastb and (
                    (dh is None and t0 + ntl == NT) or _is_lastpiece(dh)
                ):
                    epi1(b)
                    st["epi1_done"] = True
                for kk in range(ntl):
                    t = t0 + kk
                    stat = st["alp"][:, t * 4 : (t + 1) * 4]
                    nc.tensor.matmul(
                        ap0g[:],
                        stat,
                        st["encT"][:, t * D : t * D + 512],
                        start=(first and kk == 0),
                        stop=(lastb and t == NT - 1),
                    )
                    nc.tensor.matmul(
                        ap1g[:],
                        stat,
                        st["encT"][:, t * D + 512 : (t + 1) * D],
                        start=(first and kk == 0),
                        stop=(lastb and t == NT - 1),
                    )

            def epi1(b):
                """softmax denominator tiles: ones-column stationary sums
                batch b's alphas into row b of totg."""
                st = state[b]
                nc.tensor.matmul(
                    totg[:],
                    ones_pad[:, b * 4 : (b + 1) * 4].bitcast(F32R),
                    st["alp"][:],
                    start=(b == 0),
                    stop=(b == BPC - 1),
                )

            def epi_final():
                """drain the PSUM rows and ship rows+denominators in a
                single [4, 1056] DMA on the ACT ring."""
                attx_sb = asmp.tile([BPC, OW], F32, tag="asb", name="attx_sb")
                nc.scalar.activation(attx_sb[:, 0:512], ap0g[:], AF.Copy)
                if totfirst:
                    nc.vector.tensor_copy(
                        attx_sb[:, 1024 : 1024 + 4 * NT], totg[:]
                    )
                    nc.vector.tensor_copy(attx_sb[:, 512:1024], ap1g[:])
                else:
                    nc.vector.tensor_copy(attx_sb[:, 512:1024], ap1g[:])
                    nc.vector.tensor_copy(
                        attx_sb[:, 1024 : 1024 + 4 * NT], totg[:]
                    )
                nc.scalar.dma_start(attx.ap(), attx_sb[:])

            def body():
                # flat schedule with 1-chunk lookahead: exp(k) is emitted
                # after tanh(k+1) so ACT never waits on DVE's scores.
                LA = looka
                for k in range(NTOT + 1 + LA):
                    if k < NTOT:
                        load_chunk(k)
                    if LA <= k <= NTOT - 1 + LA:
                        kk = k - LA
                        b, t0, ntl, dh = CH[kk]
                        lastchunk = (dh is None and t0 + ntl == NT) or (
                            dh is not None and _is_lastpiece(dh)
                        )
                        exp_mm_chunk(kk)
                        if lastchunk and not state.get(b, {}).get("epi1_done"):
                            epi1(b)
                from contextlib import nullcontext
                with tc.high_priority() if hp_epi else nullcontext():
                    epi_final()
                state.clear()

            if loop_R is None:
                body()
            else:
                assert loop_R % bodyx == 0
                with tc.For_i(0, loop_R // bodyx, staggered_reset=stag):
                    for _ in range(bodyx):
                        body()

    nc.compile()
    return nc


def _get_nc():
    if "nc" not in _CACHE:
        _CACHE["nc"] = _build()
    return _CACHE["nc"]


def _make_in_maps(encoder_outputs, decoder_hidden, W, b):
    enc = np.ascontiguousarray(np.asarray(encoder_outputs, dtype=np.float32))
    z = np.asarray(decoder_hidden, dtype=np.float32)[-1]  # [B, D]
    W = np.asarray(W, dtype=np.float32)
    b = np.asarray(b, dtype=np.float32)

    W1 = W[:D, 0]
    W2 = W[D:, 0]
    import ml_dtypes
    w1rep = np.ascontiguousarray(
        np.broadcast_to(W1[None, :], (128, D)).astype(ml_dtypes.bfloat16)
    )
    w2t = np.ascontiguousarray(W2.reshape(128, 8))
    bb128 = np.full((128, 1), float(b[0]) / 128.0, dtype=np.float32)

    in_maps = []
    for c in range(NCORES):
        zi = z[c * BPC : (c + 1) * BPC]  # [BPC, D]
        ztc = np.ascontiguousarray(
            zi.reshape(BPC, 128, 8).transpose(1, 0, 2).reshape(128, BPC * 8)
        )
        in_maps.append(
            {
                "enc": np.ascontiguousarray(enc[c * BPC : (c + 1) * BPC]),
                "zt": ztc,
                "w1rep": w1rep,
                "w2t": w2t,
                "bb128": bb128,
            }
        )
    return in_maps


def _finalize(results):
    """Gather per-core outputs into the full [B, D] attention matrix."""
    r = np.concatenate([results[c]["attx"] for c in range(NCORES)], axis=0)
    rows = r[:, :D]
    denom = r[:, D:].sum(axis=1, dtype=np.float64)
    return (rows / denom[:, None]).astype(np.float32)


def kernel(encoder_outputs, decoder_hidden, W, b, **_):
    in_maps = _make_in_maps(encoder_outputs, decoder_hidden, W, b)
    nc = _get_nc()
    res = run_bass_kernel_spmd(nc, in_maps, list(range(NCORES)))
    return _finalize(res.results)
